# revision 1
# baseline (speedup 1.0000x reference)
# Swin-style window attention (B=256 windows, N=196, C=768, H=12) on 8 trn2 cores.
# Data-parallel over windows: 32 windows/core. Per core:
#   qT/kT = Wqk(f32r, stationary) @ x.T(bf16, moving)     [o, t] layout
#   V     = x.T(bf16, stationary) @ [Wv|0](bf16) + [vb|1] [t, o]+ones layout
#   per (window, head):
#     S.T  = kT_h.T @ qT_h (bf16, K=64, row-group packed per head parity)
#     p    = exp(0.125*S.T) * exp(rpb).T      (one ACT + one DVE op, [128,392])
#     OT/cs= [V_h | ones].T @ p               (one matmul: O.T rows 0:64,
#                                              denominator bcast rows 64:128)
#     otn  = OT * recip(cs)                   (DVE, bf16 out)
#   yT = Wp(f32r, stationary) @ O.T(bf16) + pb
# Zero on-device transposes; all contractions land on partitions naturally.
import sys

sys.path.insert(0, "/opt/trn_rl_repo")

from contextlib import ExitStack

import ml_dtypes
import numpy as np

import concourse.bass as bass
import concourse.bacc as bacc
import concourse.mybir as mybir
import concourse.tile as tile
from concourse.bass_utils import run_bass_kernel_spmd

F32 = mybir.dt.float32
F32R = mybir.dt.float32r
BF16 = mybir.dt.bfloat16
AF = mybir.ActivationFunctionType

_NC_CACHE = {}
NCORES = 8
B, N, C, H = 256, 196, 768, 12
HD = C // H  # 64
WPC = B // NCORES  # 32 windows per core
T = WPC * N  # 6272 tokens per core
CB = C // 128  # 6 contraction blocks
CHUNK_W = 4  # windows per chunk
VW = C  # V tile width (ones for the denominator live in a const tile)


def _install_ntff_hook():
    """Recreate the antenv.axon_hooks shim so trace=True works under axon."""
    import types

    if "antenv.axon_hooks" in sys.modules:
        return
    mod = types.ModuleType("antenv.axon_hooks")
    mod._hook = None
    mod.set_axon_ntff_profile_hook = lambda h: setattr(mod, "_hook", h)
    mod.get_axon_ntff_profile_hook = lambda: mod._hook
    sys.modules["antenv.axon_hooks"] = mod
    try:
        sys.path.insert(0, "/root/.axon_site/trn_agent_boot")
        from trn_boot import _ntff_profile_via_ctypes

        hook = _ntff_profile_via_ctypes("/opt/axon/libaxon_pjrt.so")
        if hook is not None:
            mod._hook = hook
    except Exception:
        pass


def _build_nc(wpc=WPC, chunk_w=CHUNK_W):
    t_total = wpc * N
    nchunk = wpc // chunk_w
    chunk_t = chunk_w * N

    nc = bacc.Bacc("TRN2", target_bir_lowering=False, debug=False,
                   num_devices=NCORES)
    xT_d = nc.dram_tensor("xT", [C, t_total], BF16, kind="ExternalInput").ap()
    wqk_d = nc.dram_tensor("wqkT", [C, 2 * C], BF16, kind="ExternalInput").ap()
    wv_d = nc.dram_tensor("wvT", [C, VW], BF16, kind="ExternalInput").ap()
    wp_d = nc.dram_tensor("projwT", [C, C], BF16, kind="ExternalInput").ap()
    qb_d = nc.dram_tensor("qbT", [128, CB], F32, kind="ExternalInput").ap()
    pb_d = nc.dram_tensor("pbT", [128, CB], F32, kind="ExternalInput").ap()
    vb_d = nc.dram_tensor("vb", [1, VW], BF16, kind="ExternalInput").ap()
    erp_d = nc.dram_tensor("erpT", [H, 128, 2 * N], BF16,
                           kind="ExternalInput").ap()
    sel_d = nc.dram_tensor("sel", [H // 2, H, 128], BF16,
                           kind="ExternalInput").ap()
    yT_d = nc.dram_tensor("yT", [C, t_total], F32, kind="ExternalOutput").ap()

    with tile.TileContext(nc) as tc, ExitStack() as ctx:
        const = ctx.enter_context(tc.tile_pool(name="const", bufs=1))
        wpool = ctx.enter_context(tc.tile_pool(name="w", bufs=1))
        xpool = ctx.enter_context(tc.tile_pool(name="x", bufs=2))
        qkpool = ctx.enter_context(tc.tile_pool(name="qk", bufs=2))
        vpool = ctx.enter_context(tc.tile_pool(name="v", bufs=2))
        otpool = ctx.enter_context(tc.tile_pool(name="ot", bufs=2))
        ppool = ctx.enter_context(tc.tile_pool(name="p", bufs=4))
        rpool = ctx.enter_context(tc.tile_pool(name="r", bufs=2))
        opool = ctx.enter_context(tc.tile_pool(name="ou", bufs=2))
        ypool = ctx.enter_context(tc.tile_pool(name="y", bufs=2))
        ps_mm = ctx.enter_context(tc.tile_pool(name="psmm", bufs=2,
                                               space="PSUM"))
        ps_st = ctx.enter_context(tc.tile_pool(name="psst", bufs=2,
                                               space="PSUM"))
        ps_ot = ctx.enter_context(tc.tile_pool(name="psot", bufs=2,
                                               space="PSUM"))

        # ---- resident constants / weights ----
        wqk, wv, wp = [], [], []
        for cb in range(CB):
            t = wpool.tile([128, 2 * C], BF16, tag=f"wqk{cb}")
            nc.sync.dma_start(t[:], wqk_d[cb * 128:(cb + 1) * 128, :])
            wqk.append(t)
            t = wpool.tile([128, VW], BF16, tag=f"wv{cb}")
            nc.sync.dma_start(t[:], wv_d[cb * 128:(cb + 1) * 128, :])
            wv.append(t)
            t = wpool.tile([128, C], BF16, tag=f"wp{cb}")
            nc.sync.dma_start(t[:], wp_d[cb * 128:(cb + 1) * 128, :])
            wp.append(t)
        erp = []
        for h in range(H):
            t = wpool.tile([128, 2 * N], BF16, tag=f"erp{h}")
            nc.sync.dma_start(t[:], erp_d[h, :, :])
            erp.append(t)
        onesrow = const.tile([1, 128], BF16)
        nc.vector.memset(onesrow[:], 1.0)
        ones128 = const.tile([128, 64], BF16)
        nc.vector.memset(ones128[:], 1.0)
        sel = []
        for j in range(H // 2):
            t = const.tile([H, 128], BF16, tag=f"sel{j}")
            nc.sync.dma_start(t[:], sel_d[j, :, :])
            sel.append(t)
        qb = const.tile([128, CB], F32)
        nc.sync.dma_start(qb[:], qb_d[:, :])
        pb = const.tile([128, CB], F32)
        nc.sync.dma_start(pb[:], pb_d[:, :])
        vb = const.tile([1, VW], BF16)
        nc.sync.dma_start(vb[:], vb_d[:, :])

        fin_pending = None
        proj_pending = None

        for ch in range(nchunk):
            t0 = ch * chunk_t
            xt = []
            for cb in range(CB):
                t = xpool.tile([128, chunk_t], BF16, tag=f"xt{cb}")
                nc.sync.dma_start(t[:], xT_d[cb * 128:(cb + 1) * 128,
                                             t0:t0 + chunk_t])
                xt.append(t)

            tslices = [(i * 512, min(512, chunk_t - i * 512))
                       for i in range((chunk_t + 511) // 512)]

            # ---- Q.T / K.T ----
            qT, kT = [], []
            for ob in range(CB):
                t = qkpool.tile([128, chunk_t], BF16, tag=f"qT{ob}")
                qT.append(t)
            for ob in range(CB):
                t = qkpool.tile([128, chunk_t + 64], BF16, tag=f"kT{ob}")
                nc.vector.memset(t[:, chunk_t:chunk_t + 64], 0.0)
                kT.append(t)
            for ob in range(2 * CB):
                dst = qT[ob] if ob < CB else kT[ob - CB]
                o = ob * 128
                pt = ps_mm.tile([128, chunk_t], F32, tag="mm")
                for cb in range(CB):
                    for (ts, tl) in tslices:
                        nc.tensor.matmul(
                            pt[:, ts:ts + tl],
                            wqk[cb][:, o:o + 128],
                            xt[cb][:, ts:ts + tl],
                            start=(cb == 0), stop=(cb == CB - 1))
                if ob < CB:  # q: bias here, softmax scale folded into exp
                    nc.scalar.activation(dst[:], pt[:, 0:chunk_t], AF.Identity,
                                         bias=qb[:, ob:ob + 1])
                else:  # k: plain copy/cast
                    nc.scalar.copy(dst[:, 0:chunk_t], pt[:, 0:chunk_t])

            if fin_pending is not None:
                finish_window(*fin_pending)
                fin_pending = None
            if proj_pending is not None:
                proj_pending()
                proj_pending = None

            # ---- V (+vb) token-major, with trailing ones block ----
            vtiles = []
            for w in range(chunk_w):
                wrow = []
                for (moff, mlen) in ((0, 128), (128, 68)):
                    trel = w * N + moff
                    vt = vpool.tile([128, VW], BF16, tag=f"vb{w}_{moff}")
                    pv = ps_mm.tile([128, chunk_t], F32, tag="mm")
                    for (noff, nlen) in ((0, 512), (512, C - 512)):
                        nc.tensor.matmul(
                            pv[0:mlen, noff:noff + nlen],
                            onesrow[:, 0:mlen],
                            vb[:, noff:noff + nlen],
                            start=True, stop=False)
                    for cb in range(CB):
                        for (noff, nlen) in ((0, 512), (512, C - 512)):
                            nc.tensor.matmul(
                                pv[0:mlen, noff:noff + nlen],
                                xt[cb][:, trel:trel + mlen],
                                wv[cb][:, noff:noff + nlen],
                                start=False, stop=(cb == CB - 1))
                    nc.vector.tensor_copy(vt[0:mlen, :], pv[0:mlen, 0:VW])
                    wrow.append(vt)
                vtiles.append(wrow)

            # ---- attention per (window, head) ----
            ot_sb = []
            for ob in range(CB):
                t = otpool.tile([128, chunk_t], BF16, tag=f"ot{ob}")
                ot_sb.append(t)
            def finish_window(wq_tok, otu, recb, ot_sb):
                for hp in range(H // 2):
                    h0 = 2 * hp
                    ob = h0 // 2
                    rb = ps_ot.tile([128, N], F32, tag="ot")
                    nc.tensor.matmul(rb[:], sel[hp][:], recb[:],
                                     start=True, stop=True)
                    for hi in range(2):
                        h = h0 + hi
                        prt = (h % 2) * 64
                        nc.vector.tensor_mul(
                            ot_sb[ob][prt:prt + 64, wq_tok:wq_tok + N],
                            rb[prt:prt + 64, :],
                            otu[0:64, h * N:(h + 1) * N])

            for w in range(chunk_w):
                wq_tok = w * N
                den = rpool.tile([H, N], F32, tag="den")
                otu = opool.tile([65, H * N], F32, tag="otun")
                for hp in range(H // 2):
                    h0, h1 = 2 * hp, 2 * hp + 1
                    ob = h0 // 2
                    sts, ps_ = [], []
                    for hi, h in enumerate((h0, h1)):
                        prt = (h % 2) * 64
                        qh = qT[ob][prt:prt + 64, wq_tok:wq_tok + N]
                        st = ps_st.tile([128, 2 * N], F32, tag="st")
                        nc.tensor.matmul(
                            st[:, 0:N],
                            kT[ob][prt:prt + 64, wq_tok:wq_tok + 128],
                            qh, start=True, stop=True)
                        nc.tensor.matmul(
                            st[:, N:2 * N],
                            kT[ob][prt:prt + 64, wq_tok + 128:wq_tok + 256],
                            qh, start=True, stop=True)
                        sts.append(st)
                    for hi, h in enumerate((h0, h1)):
                        p = ppool.tile([128, 2 * N], BF16, tag="p")
                        nc.scalar.activation(p[:], sts[hi][:], AF.Exp,
                                             scale=0.125)
                        nc.vector.tensor_mul(p[:], p[:], erp[h][:])
                        ps_.append(p)
                    for hi, h in enumerate((h0, h1)):
                        p = ps_[hi]
                        ot = ps_ot.tile([128, N], F32, tag="ot")
                        for bi, (moff, mlen) in enumerate(((0, 128),
                                                          (128, 68))):
                            nc.tensor.matmul(
                                ot[0:64, :],
                                vtiles[w][bi][0:mlen, h * 64:h * 64 + 64],
                                p[0:mlen, bi * N:(bi + 1) * N],
                                start=(bi == 0), stop=(bi == 1))
                        for bi, (moff, mlen) in enumerate(((0, 128),
                                                          (128, 68))):
                            nc.tensor.matmul(
                                ot[64:128, :], ones128[0:mlen, :],
                                p[0:mlen, bi * N:(bi + 1) * N],
                                start=(bi == 0), stop=(bi == 1),
                                tile_position=(0, 64))
                        nc.vector.tensor_copy(
                            otu[:, h * N:(h + 1) * N], ot[0:65, :])
                srcrow = otu[64:65, 0:H * N]
                nc.gpsimd.dma_start(
                    den[:], bass.AP(srcrow.tensor, srcrow.offset,
                                    [srcrow.ap[0], [N, H], [1, N]]))
                rec = rpool.tile([H, N], F32, tag="rec")
                nc.vector.reciprocal(rec[:], den[:])
                recb = rpool.tile([H, N], BF16, tag="recb")
                nc.vector.tensor_copy(recb[:], rec[:])
                if fin_pending is not None:
                    finish_window(*fin_pending)
                fin_pending = (wq_tok, otu, recb, ot_sb)
            # ---- proj (deferred one chunk) ----
            def make_proj(t0, ot_sb, yts):
                def emit_proj():
                    for opb in range(CB):
                        o = opb * 128
                        pt = ps_mm.tile([128, chunk_t], F32, tag="mm")
                        for (ts, tl) in yts:
                            for ob in range(CB):
                                nc.tensor.matmul(
                                    pt[:, ts:ts + tl],
                                    wp[ob][:, o:o + 128],
                                    ot_sb[ob][:, ts:ts + tl],
                                    start=(ob == 0), stop=(ob == CB - 1))
                        yt = ypool.tile([128, chunk_t], F32, tag="y")
                        nc.scalar.activation(yt[:], pt[:, 0:chunk_t],
                                             AF.Identity,
                                             bias=pb[:, opb:opb + 1])
                        nc.sync.dma_start(yT_d[o:o + 128, t0:t0 + chunk_t],
                                          yt[:])
                return emit_proj
            proj_pending = make_proj(t0, ot_sb, tslices)

        if fin_pending is not None:
            finish_window(*fin_pending)
        if proj_pending is not None:
            proj_pending()

    nc.compile()
    return nc


def _host_prep(x, qkv_w, q_bias, v_bias, rpb_table, proj_w, proj_b, rel_index,
               wpc=WPC):
    x = np.asarray(x, np.float32)
    ncores = x.shape[0] // wpc
    t_total = wpc * N
    xT = np.ascontiguousarray(
        x.reshape(ncores, t_total, C).transpose(0, 2, 1)).astype(
            ml_dtypes.bfloat16)
    qkv_w = np.asarray(qkv_w, np.float32)
    wqkT = np.ascontiguousarray(qkv_w[0:2 * C].T).astype(ml_dtypes.bfloat16)
    wvT = np.ascontiguousarray(qkv_w[2 * C:3 * C].T).astype(
        ml_dtypes.bfloat16)
    projwT = np.ascontiguousarray(
        np.asarray(proj_w, np.float32).T).astype(ml_dtypes.bfloat16)
    qbT = np.ascontiguousarray(
        np.asarray(q_bias, np.float32).reshape(CB, 128).T)
    pbT = np.ascontiguousarray(
        np.asarray(proj_b, np.float32).reshape(CB, 128).T)
    vb = np.asarray(v_bias, np.float32).reshape(1, C).astype(
        ml_dtypes.bfloat16)
    rel = np.asarray(rel_index).reshape(N, N)
    rpb = np.asarray(rpb_table, np.float32)[rel]              # [n, m, H]
    erp_full = np.exp(rpb).transpose(2, 1, 0)                 # [H, m, n]
    erpT = np.zeros((H, 128, 2 * N), np.float32)
    erpT[:, :, :N] = erp_full[:, 0:128, :]
    erpT[:, 0:68, N:] = erp_full[:, 128:196, :]
    erpT = erpT.astype(ml_dtypes.bfloat16)
    sel = np.zeros((H // 2, H, 128), np.float32)
    for j in range(H // 2):
        sel[j, 2 * j, 0:64] = 1.0
        sel[j, 2 * j + 1, 64:128] = 1.0
    sel = sel.astype(ml_dtypes.bfloat16)
    return xT, wqkT, wvT, projwT, qbT, pbT, vb, erpT, sel


def kernel(x, qkv_w, q_bias, v_bias, rpb_table, proj_w, proj_b, rel_index,
           num_heads=12, _trace=False):
    xT, wqkT, wvT, projwT, qbT, pbT, vb, erpT, sel = _host_prep(
        x, qkv_w, q_bias, v_bias, rpb_table, proj_w, proj_b, rel_index)
    if _trace:
        _install_ntff_hook()
    nc = _NC_CACHE.get("nc")
    if nc is None:
        nc = _build_nc()
        _NC_CACHE["nc"] = nc
    in_maps = [
        {"xT": np.ascontiguousarray(xT[c]), "wqkT": wqkT, "wvT": wvT,
         "projwT": projwT, "qbT": qbT, "pbT": pbT, "vb": vb, "erpT": erpT,
         "sel": sel}
        for c in range(NCORES)
    ]
    res = run_bass_kernel_spmd(nc, in_maps, core_ids=list(range(NCORES)),
                               trace=_trace)
    yT = np.stack([res.results[c]["yT"] for c in range(NCORES)])
    out = np.ascontiguousarray(yT.transpose(0, 2, 1)).reshape(B, N, C)
    if _trace:
        kernel._last_exec_time_ns = res.exec_time_ns
        kernel._last_results = res
    return out.astype(np.float32)



# revision 10
# speedup vs baseline: 1.0027x; 1.0027x over previous
# Swin-style window attention (B=256 windows, N=196, C=768, H=12) on 8 trn2 cores.
# Data-parallel over windows: 32 windows/core. Per core:
#   qT/kT = Wqk(f32r, stationary) @ x.T(bf16, moving)     [o, t] layout
#   V     = x.T(bf16, stationary) @ [Wv|0](bf16) + [vb|1] [t, o]+ones layout
#   per (window, head):
#     S.T  = kT_h.T @ qT_h (bf16, K=64, row-group packed per head parity)
#     p    = exp(0.125*S.T) * exp(rpb).T      (one ACT + one DVE op, [128,392])
#     OT/cs= [V_h | ones].T @ p               (one matmul: O.T rows 0:64,
#                                              denominator bcast rows 64:128)
#     otn  = OT * recip(cs)                   (DVE, bf16 out)
#   yT = Wp(f32r, stationary) @ O.T(bf16) + pb
# Zero on-device transposes; all contractions land on partitions naturally.
import sys

sys.path.insert(0, "/opt/trn_rl_repo")

from contextlib import ExitStack

import ml_dtypes
import numpy as np

import concourse.bass as bass
import concourse.bacc as bacc
import concourse.mybir as mybir
import concourse.tile as tile
from concourse.bass_utils import run_bass_kernel_spmd

F32 = mybir.dt.float32
F32R = mybir.dt.float32r
BF16 = mybir.dt.bfloat16
AF = mybir.ActivationFunctionType

_NC_CACHE = {}
NCORES = 8
B, N, C, H = 256, 196, 768, 12
HD = C // H  # 64
WPC = B // NCORES  # 32 windows per core
T = WPC * N  # 6272 tokens per core
CB = C // 128  # 6 contraction blocks
CHUNK_W = 4  # windows per chunk
VW = H * (HD + 1)  # V tile width: per head [V_h | ones-col] for the denominator


def _install_ntff_hook():
    """Recreate the antenv.axon_hooks shim so trace=True works under axon."""
    import types

    if "antenv.axon_hooks" in sys.modules:
        return
    mod = types.ModuleType("antenv.axon_hooks")
    mod._hook = None
    mod.set_axon_ntff_profile_hook = lambda h: setattr(mod, "_hook", h)
    mod.get_axon_ntff_profile_hook = lambda: mod._hook
    sys.modules["antenv.axon_hooks"] = mod
    try:
        sys.path.insert(0, "/root/.axon_site/trn_agent_boot")
        from trn_boot import _ntff_profile_via_ctypes

        hook = _ntff_profile_via_ctypes("/opt/axon/libaxon_pjrt.so")
        if hook is not None:
            mod._hook = hook
    except Exception:
        pass


def _build_nc(wpc=WPC, chunk_w=CHUNK_W):
    t_total = wpc * N
    nchunk = wpc // chunk_w
    chunk_t = chunk_w * N

    nc = bacc.Bacc("TRN2", target_bir_lowering=False, debug=False,
                   num_devices=NCORES)
    xT_d = nc.dram_tensor("xT", [C, t_total], BF16, kind="ExternalInput").ap()
    wqk_d = nc.dram_tensor("wqkT", [C, 2 * C], BF16, kind="ExternalInput").ap()
    wv_d = nc.dram_tensor("wvT", [C, VW], BF16, kind="ExternalInput").ap()
    wp_d = nc.dram_tensor("projwT", [C, C], BF16, kind="ExternalInput").ap()
    qb_d = nc.dram_tensor("qbT", [128, CB], F32, kind="ExternalInput").ap()
    pb_d = nc.dram_tensor("pbT", [128, CB], F32, kind="ExternalInput").ap()
    erp_d = nc.dram_tensor("erpT", [H, 128, 2 * N], BF16,
                           kind="ExternalInput").ap()
    sel_d = nc.dram_tensor("sel", [H // 2, H, 128], BF16,
                           kind="ExternalInput").ap()
    yT_d = nc.dram_tensor("yT", [C, t_total], F32, kind="ExternalOutput").ap()

    with tile.TileContext(nc) as tc, ExitStack() as ctx:
        const = ctx.enter_context(tc.tile_pool(name="const", bufs=1))
        wpool = ctx.enter_context(tc.tile_pool(name="w", bufs=1))
        xpool = ctx.enter_context(tc.tile_pool(name="x", bufs=2))
        qkpool = ctx.enter_context(tc.tile_pool(name="qk", bufs=2))
        vpool = ctx.enter_context(tc.tile_pool(name="v", bufs=2))
        otpool = ctx.enter_context(tc.tile_pool(name="ot", bufs=2))
        ppool = ctx.enter_context(tc.tile_pool(name="p", bufs=4))
        rpool = ctx.enter_context(tc.tile_pool(name="r", bufs=2))
        opool = ctx.enter_context(tc.tile_pool(name="ou", bufs=2))
        ypool = ctx.enter_context(tc.tile_pool(name="y", bufs=2))
        ps_mm = ctx.enter_context(tc.tile_pool(name="psmm", bufs=2,
                                               space="PSUM"))
        ps_st = ctx.enter_context(tc.tile_pool(name="psst", bufs=2,
                                               space="PSUM"))
        ps_ot = ctx.enter_context(tc.tile_pool(name="psot", bufs=2,
                                               space="PSUM"))

        # ---- resident constants / weights ----
        wqk, wv, wp = [], [], []
        for cb in range(CB):
            t = wpool.tile([128, 2 * C], BF16, tag=f"wqk{cb}")
            nc.sync.dma_start(t[:], wqk_d[cb * 128:(cb + 1) * 128, :])
            wqk.append(t)
            t = wpool.tile([128, VW], BF16, tag=f"wv{cb}")
            nc.sync.dma_start(t[:], wv_d[cb * 128:(cb + 1) * 128, :])
            wv.append(t)
            t = wpool.tile([128, C], BF16, tag=f"wp{cb}")
            nc.sync.dma_start(t[:], wp_d[cb * 128:(cb + 1) * 128, :])
            wp.append(t)
        erp = []
        for h in range(H):
            t = wpool.tile([128, 2 * N], BF16, tag=f"erp{h}")
            nc.sync.dma_start(t[:], erp_d[h, :, :])
            erp.append(t)
        vmask = const.tile([128, VW], BF16)
        nc.vector.memset(vmask[:], 0.0)
        for h in range(H):
            nc.vector.memset(vmask[:, h * 65 + 64:h * 65 + 65], 1.0)
        sel = []
        for j in range(H // 2):
            t = const.tile([H, 128], BF16, tag=f"sel{j}")
            nc.sync.dma_start(t[:], sel_d[j, :, :])
            sel.append(t)
        qb = const.tile([128, CB], F32)
        nc.sync.dma_start(qb[:], qb_d[:, :])
        pb = const.tile([128, CB], F32)
        nc.sync.dma_start(pb[:], pb_d[:, :])

        fin_pending = None
        proj_pending = None

        for ch in range(nchunk):
            t0 = ch * chunk_t
            xt = []
            for cb in range(CB):
                t = xpool.tile([128, chunk_t], BF16, tag=f"xt{cb}")
                nc.sync.dma_start(t[:], xT_d[cb * 128:(cb + 1) * 128,
                                             t0:t0 + chunk_t])
                xt.append(t)

            tslices = [(i * 512, min(512, chunk_t - i * 512))
                       for i in range((chunk_t + 511) // 512)]

            # ---- Q.T / K.T ----
            qT, kT = [], []
            for ob in range(CB):
                t = qkpool.tile([128, chunk_t], BF16, tag=f"qT{ob}")
                qT.append(t)
            for ob in range(CB):
                t = qkpool.tile([128, chunk_t + 64], BF16, tag=f"kT{ob}")
                nc.vector.memset(t[:, chunk_t:chunk_t + 64], 0.0)
                kT.append(t)
            for ob in range(2 * CB):
                dst = qT[ob] if ob < CB else kT[ob - CB]
                o = ob * 128
                pt = ps_mm.tile([128, chunk_t], F32, tag="mm")
                for cb in range(CB):
                    for (ts, tl) in tslices:
                        nc.tensor.matmul(
                            pt[:, ts:ts + tl],
                            wqk[cb][:, o:o + 128],
                            xt[cb][:, ts:ts + tl],
                            start=(cb == 0), stop=(cb == CB - 1))
                if ob < CB:  # q: bias here, softmax scale folded into exp
                    nc.scalar.activation(dst[:], pt[:, 0:chunk_t], AF.Identity,
                                         bias=qb[:, ob:ob + 1])
                else:  # k: plain copy/cast
                    nc.scalar.copy(dst[:, 0:chunk_t], pt[:, 0:chunk_t])

            if fin_pending is not None:
                finish_window(*fin_pending)
                fin_pending = None
            if proj_pending is not None:
                proj_pending()
                proj_pending = None

            # ---- V token-major as [V_h | ones-col] per head (ones via mask
            # add in the PSUM->SBUF copy; v_bias folded into proj bias) ----
            vtiles = []
            for w in range(chunk_w):
                wrow = []
                for (moff, mlen) in ((0, 128), (128, 68)):
                    trel = w * N + moff
                    vt = vpool.tile([128, VW], BF16, tag=f"vb{w}_{moff}")
                    pv = ps_mm.tile([128, chunk_t], F32, tag="mm")
                    for cb in range(CB):
                        for (noff, nlen) in ((0, 512), (512, VW - 512)):
                            nc.tensor.matmul(
                                pv[0:mlen, noff:noff + nlen],
                                xt[cb][:, trel:trel + mlen],
                                wv[cb][:, noff:noff + nlen],
                                start=(cb == 0), stop=(cb == CB - 1))
                    nc.vector.tensor_add(vt[0:mlen, :], pv[0:mlen, 0:VW],
                                         vmask[0:mlen, :])
                    wrow.append(vt)
                vtiles.append(wrow)

            # ---- attention per (window, head) ----
            ot_sb = []
            for ob in range(CB):
                t = otpool.tile([128, chunk_t], BF16, tag=f"ot{ob}")
                ot_sb.append(t)
            def finish_window(wq_tok, otu, recb, ot_sb):
                for hp in range(H // 2):
                    h0 = 2 * hp
                    ob = h0 // 2
                    rb = ps_ot.tile([128, N], F32, tag="ot")
                    nc.tensor.matmul(rb[:], sel[hp][:], recb[:],
                                     start=True, stop=True)
                    for hi in range(2):
                        h = h0 + hi
                        prt = (h % 2) * 64
                        nc.vector.tensor_mul(
                            ot_sb[ob][prt:prt + 64, wq_tok:wq_tok + N],
                            rb[prt:prt + 64, :],
                            otu[0:64, h * N:(h + 1) * N])

            for w in range(chunk_w):
                wq_tok = w * N
                den = rpool.tile([H, N], F32, tag="den")
                otu = opool.tile([65, H * N], F32, tag="otun")
                for hp in range(H // 2):
                    h0, h1 = 2 * hp, 2 * hp + 1
                    ob = h0 // 2
                    sts, ps_ = [], []
                    for hi, h in enumerate((h0, h1)):
                        prt = (h % 2) * 64
                        qh = qT[ob][prt:prt + 64, wq_tok:wq_tok + N]
                        st = ps_st.tile([128, 2 * N], F32, tag="st")
                        nc.tensor.matmul(
                            st[:, 0:N],
                            kT[ob][prt:prt + 64, wq_tok:wq_tok + 128],
                            qh, start=True, stop=True)
                        nc.tensor.matmul(
                            st[:, N:2 * N],
                            kT[ob][prt:prt + 64, wq_tok + 128:wq_tok + 256],
                            qh, start=True, stop=True)
                        sts.append(st)
                    for hi, h in enumerate((h0, h1)):
                        p = ppool.tile([128, 2 * N], BF16, tag="p")
                        nc.scalar.activation(p[:], sts[hi][:], AF.Exp,
                                             scale=0.125)
                        nc.vector.tensor_mul(p[:], p[:], erp[h][:])
                        ps_.append(p)
                    for hi, h in enumerate((h0, h1)):
                        p = ps_[hi]
                        ot = ps_ot.tile([128, N], F32, tag="ot")
                        for bi, (moff, mlen) in enumerate(((0, 128),
                                                          (128, 68))):
                            nc.tensor.matmul(
                                ot[0:65, :],
                                vtiles[w][bi][0:mlen, h * 65:h * 65 + 65],
                                p[0:mlen, bi * N:(bi + 1) * N],
                                start=(bi == 0), stop=(bi == 1))
                        nc.vector.tensor_copy(
                            otu[:, h * N:(h + 1) * N], ot[0:65, :])
                srcrow = otu[64:65, 0:H * N]
                nc.gpsimd.dma_start(
                    den[:], bass.AP(srcrow.tensor, srcrow.offset,
                                    [srcrow.ap[0], [N, H], [1, N]]))
                rec = rpool.tile([H, N], F32, tag="rec")
                nc.vector.reciprocal(rec[:], den[:])
                recb = rpool.tile([H, N], BF16, tag="recb")
                nc.vector.tensor_copy(recb[:], rec[:])
                if fin_pending is not None:
                    finish_window(*fin_pending)
                fin_pending = (wq_tok, otu, recb, ot_sb)
            # ---- proj (deferred one chunk) ----
            def make_proj(t0, ot_sb, yts):
                def emit_proj():
                    for opb in range(CB):
                        o = opb * 128
                        pt = ps_mm.tile([128, chunk_t], F32, tag="mm")
                        for (ts, tl) in yts:
                            for ob in range(CB):
                                nc.tensor.matmul(
                                    pt[:, ts:ts + tl],
                                    wp[ob][:, o:o + 128],
                                    ot_sb[ob][:, ts:ts + tl],
                                    start=(ob == 0), stop=(ob == CB - 1))
                        yt = ypool.tile([128, chunk_t], F32, tag="y")
                        nc.scalar.activation(yt[:], pt[:, 0:chunk_t],
                                             AF.Identity,
                                             bias=pb[:, opb:opb + 1])
                        nc.sync.dma_start(yT_d[o:o + 128, t0:t0 + chunk_t],
                                          yt[:])
                return emit_proj
            proj_pending = make_proj(t0, ot_sb, tslices)

        if fin_pending is not None:
            finish_window(*fin_pending)
        if proj_pending is not None:
            proj_pending()

    nc.compile()
    return nc


def _host_prep(x, qkv_w, q_bias, v_bias, rpb_table, proj_w, proj_b, rel_index,
               wpc=WPC):
    x = np.asarray(x, np.float32)
    ncores = x.shape[0] // wpc
    t_total = wpc * N
    xT = np.ascontiguousarray(
        x.reshape(ncores, t_total, C).transpose(0, 2, 1)).astype(
            ml_dtypes.bfloat16)
    qkv_w = np.asarray(qkv_w, np.float32)
    wqkT = np.ascontiguousarray(qkv_w[0:2 * C].T).astype(ml_dtypes.bfloat16)
    wvT_base = qkv_w[2 * C:3 * C].T                           # [C, C]
    wvT = np.zeros((C, VW), np.float32)
    for h in range(H):
        wvT[:, h * 65:h * 65 + 64] = wvT_base[:, h * 64:(h + 1) * 64]
    wvT = np.ascontiguousarray(wvT).astype(ml_dtypes.bfloat16)
    projwT = np.ascontiguousarray(
        np.asarray(proj_w, np.float32).T).astype(ml_dtypes.bfloat16)
    qbT = np.ascontiguousarray(
        np.asarray(q_bias, np.float32).reshape(CB, 128).T)
    # v_bias folds into the proj bias: softmax rows sum to 1, so
    # P@(V+vb) = P@V + vb and y += vb @ proj_w.T
    pb_eff = (np.asarray(proj_b, np.float32) +
              np.asarray(v_bias, np.float32) @
              np.asarray(proj_w, np.float32).T)
    pbT = np.ascontiguousarray(pb_eff.reshape(CB, 128).T)
    rel = np.asarray(rel_index).reshape(N, N)
    rpb = np.asarray(rpb_table, np.float32)[rel]              # [n, m, H]
    erp_full = np.exp(rpb).transpose(2, 1, 0)                 # [H, m, n]
    erpT = np.zeros((H, 128, 2 * N), np.float32)
    erpT[:, :, :N] = erp_full[:, 0:128, :]
    erpT[:, 0:68, N:] = erp_full[:, 128:196, :]
    erpT = erpT.astype(ml_dtypes.bfloat16)
    sel = np.zeros((H // 2, H, 128), np.float32)
    for j in range(H // 2):
        sel[j, 2 * j, 0:64] = 1.0
        sel[j, 2 * j + 1, 64:128] = 1.0
    sel = sel.astype(ml_dtypes.bfloat16)
    return xT, wqkT, wvT, projwT, qbT, pbT, erpT, sel


def kernel(x, qkv_w, q_bias, v_bias, rpb_table, proj_w, proj_b, rel_index,
           num_heads=12, _trace=False):
    xT, wqkT, wvT, projwT, qbT, pbT, erpT, sel = _host_prep(
        x, qkv_w, q_bias, v_bias, rpb_table, proj_w, proj_b, rel_index)
    if _trace:
        _install_ntff_hook()
    nc = _NC_CACHE.get("nc")
    if nc is None:
        nc = _build_nc()
        _NC_CACHE["nc"] = nc
    in_maps = [
        {"xT": np.ascontiguousarray(xT[c]), "wqkT": wqkT, "wvT": wvT,
         "projwT": projwT, "qbT": qbT, "pbT": pbT, "erpT": erpT,
         "sel": sel}
        for c in range(NCORES)
    ]
    res = run_bass_kernel_spmd(nc, in_maps, core_ids=list(range(NCORES)),
                               trace=_trace)
    yT = np.stack([res.results[c]["yT"] for c in range(NCORES)])
    out = np.ascontiguousarray(yT.transpose(0, 2, 1)).reshape(B, N, C)
    if _trace:
        kernel._last_exec_time_ns = res.exec_time_ns
        kernel._last_results = res
    return out.astype(np.float32)

